# revision 1
# baseline (speedup 1.0000x reference)
"""Self-contained kernel for nn_Attention_85873576116895 (sparse_attention).

Implements: GroupNorm(1,C) -> qkv 1x1 conv -> 8x8 windowed local attention ->
residual + GroupNorm -> 2x2 patch-merge (LayerNorm + reduction matmul) ->
global attention (full-res Q attends to pooled KV) -> residual -> output proj.

Distribution: data-parallel over batch B=4 across the first 4 NeuronCores
(each sample's compute is independent end-to-end; GroupNorm(1,C) is
per-sample so no cross-core reduction is needed). Falls back to a single
device, then to host CPU, if multi-device dispatch fails.
"""

import numpy as np

DIM = 256
HEAD_DIM = 32
GRID = 8
EPS_GN = 1e-6
EPS_LN = 1e-5

_COMPILED = {}


def _build():
    import jax
    import jax.numpy as jnp

    def groupnorm1(x, w, b):
        m = jnp.mean(x, axis=(1, 2, 3), keepdims=True)
        v = jnp.var(x, axis=(1, 2, 3), keepdims=True)
        xn = (x - m) * jax.lax.rsqrt(v + EPS_GN)
        return xn * w[None, :, None, None] + b[None, :, None, None]

    def conv1x1(x, w, b=None):
        y = jnp.einsum('bchw,oc->bohw', x, w)
        if b is not None:
            y = y + b[None, :, None, None]
        return y

    def fwd(x, norm_w, norm_b, qkv_w, qkv_b, proj_w, proj_b,
            grid_norm_w, grid_norm_b, pm_ln_w, pm_ln_b, pm_red_w,
            ds_norm_w, ds_norm_b, q_w, q_b, kv_w, kv_b):
        B, C, H, W = x.shape
        hd = HEAD_DIM
        nh = C // hd
        gs = GRID
        gh, gw = H // gs, W // gs
        scale = hd ** -0.5

        # ---- local windowed (grid) attention ----
        qkv = conv1x1(groupnorm1(x, norm_w, norm_b), qkv_w, qkv_b)
        qkv = qkv.reshape(B, 3, nh, hd, gh, gs, gw, gs)
        qkv = qkv.transpose(1, 0, 2, 4, 6, 5, 7, 3).reshape(3, -1, gs * gs, hd)
        q, k, v = qkv[0], qkv[1], qkv[2]
        attn = jax.nn.softmax(jnp.einsum('nqd,nkd->nqk', q * scale, k), axis=-1)
        gx = jnp.einsum('nqk,nkd->nqd', attn, v)
        gx = gx.reshape(B, nh, gh, gw, gs, gs, hd)
        gx = gx.transpose(0, 1, 6, 2, 4, 3, 5).reshape(B, C, H, W)
        grid_x = groupnorm1(x + gx, grid_norm_w, grid_norm_b)

        # ---- patch-merging pool (2x downsample) ----
        t = grid_x.transpose(0, 2, 3, 1)
        m = jnp.concatenate([t[:, 0::2, 0::2], t[:, 1::2, 0::2],
                             t[:, 0::2, 1::2], t[:, 1::2, 1::2]], axis=-1)
        m = m.reshape(B, (H // 2) * (W // 2), 4 * C)
        mu = jnp.mean(m, axis=-1, keepdims=True)
        var = jnp.var(m, axis=-1, keepdims=True)
        m = (m - mu) * jax.lax.rsqrt(var + EPS_LN) * pm_ln_w + pm_ln_b
        m = m @ pm_red_w.T
        pm = m.reshape(B, H // 2, W // 2, C).transpose(0, 3, 1, 2)

        # ---- global attention (full q attends to pooled kv) ----
        ds = groupnorm1(pm, ds_norm_w, ds_norm_b)
        qg = conv1x1(grid_x, q_w, q_b).reshape(B, nh, hd, H * W).transpose(0, 1, 3, 2)
        kvg = conv1x1(ds, kv_w, kv_b).reshape(B, 2, nh, hd, (H // 2) * (W // 2))
        kvg = kvg.transpose(1, 0, 2, 4, 3)
        kg, vg = kvg[0], kvg[1]
        attn2 = jax.nn.softmax(jnp.einsum('bnqd,bnkd->bnqk', qg * scale, kg), axis=-1)
        go = jnp.einsum('bnqk,bnkd->bnqd', attn2, vg)
        go = go.transpose(0, 1, 3, 2).reshape(B, C, H, W)
        global_x = go + grid_x

        return conv1x1(global_x, proj_w, proj_b)

    return jax, jnp, jax.jit(fwd)


def _numpy_fallback(**inputs):
    x = inputs['x']
    B, C, H, W = x.shape
    hd, nh, gs = HEAD_DIM, DIM // HEAD_DIM, GRID
    gh, gw = H // gs, W // gs
    scale = hd ** -0.5

    def gn(x, w, b):
        m = x.mean(axis=(1, 2, 3), keepdims=True)
        v = x.var(axis=(1, 2, 3), keepdims=True)
        return (x - m) / np.sqrt(v + EPS_GN) * w[None, :, None, None] + b[None, :, None, None]

    def conv(x, w, b=None):
        y = np.einsum('bchw,oc->bohw', x, w)
        if b is not None:
            y = y + b[None, :, None, None]
        return y

    def softmax(s):
        s = s - s.max(axis=-1, keepdims=True)
        e = np.exp(s)
        return e / e.sum(axis=-1, keepdims=True)

    i = inputs
    qkv = conv(gn(x, i['norm_w'], i['norm_b']), i['qkv_w'], i['qkv_b'])
    qkv = qkv.reshape(B, 3, nh, hd, gh, gs, gw, gs)
    qkv = qkv.transpose(1, 0, 2, 4, 6, 5, 7, 3).reshape(3, -1, gs * gs, hd)
    q, k, v = qkv[0], qkv[1], qkv[2]
    attn = softmax(np.einsum('nqd,nkd->nqk', q * scale, k))
    gx = np.einsum('nqk,nkd->nqd', attn, v)
    gx = gx.reshape(B, nh, gh, gw, gs, gs, hd)
    gx = gx.transpose(0, 1, 6, 2, 4, 3, 5).reshape(B, C, H, W)
    grid_x = gn(x + gx, i['grid_norm_w'], i['grid_norm_b'])

    t = grid_x.transpose(0, 2, 3, 1)
    m = np.concatenate([t[:, 0::2, 0::2], t[:, 1::2, 0::2],
                        t[:, 0::2, 1::2], t[:, 1::2, 1::2]], axis=-1)
    m = m.reshape(B, (H // 2) * (W // 2), 4 * C)
    mu = m.mean(axis=-1, keepdims=True)
    var = m.var(axis=-1, keepdims=True)
    m = (m - mu) / np.sqrt(var + EPS_LN) * i['pm_ln_w'] + i['pm_ln_b']
    m = m @ i['pm_red_w'].T
    pm = m.reshape(B, H // 2, W // 2, DIM).transpose(0, 3, 1, 2)

    ds = gn(pm, i['ds_norm_w'], i['ds_norm_b'])
    qg = conv(grid_x, i['q_w'], i['q_b']).reshape(B, nh, hd, H * W).transpose(0, 1, 3, 2)
    kvg = conv(ds, i['kv_w'], i['kv_b']).reshape(B, 2, nh, hd, (H // 2) * (W // 2))
    kvg = kvg.transpose(1, 0, 2, 4, 3)
    kg, vg = kvg[0], kvg[1]
    attn2 = softmax(np.einsum('bnqd,bnkd->bnqk', qg * scale, kg))
    go = np.einsum('bnqk,bnkd->bnqd', attn2, vg)
    go = go.transpose(0, 1, 3, 2).reshape(B, C, H, W)
    return conv(go + grid_x, i['proj_w'], i['proj_b']).astype(np.float32)


_ORDER = ['x', 'norm_w', 'norm_b', 'qkv_w', 'qkv_b', 'proj_w', 'proj_b',
          'grid_norm_w', 'grid_norm_b', 'pm_ln_w', 'pm_ln_b', 'pm_red_w',
          'ds_norm_w', 'ds_norm_b', 'q_w', 'q_b', 'kv_w', 'kv_b']


def kernel(**inputs) -> np.ndarray:
    try:
        if 'f' not in _COMPILED:
            _COMPILED['jax'], _COMPILED['jnp'], _COMPILED['f'] = _build()
        jax = _COMPILED['jax']
        f = _COMPILED['f']
        args = [np.asarray(inputs[k]) for k in _ORDER]
        devs = jax.devices()
        x = args[0]
        B = x.shape[0]
        try:
            # Data-parallel over batch: one sample per core on the first B cores.
            ndev = min(B, len(devs))
            if ndev >= 2 and B % ndev == 0:
                per = B // ndev
                weights = [[jax.device_put(a, devs[d]) for a in args[1:]]
                           for d in range(ndev)]
                outs = []
                for d in range(ndev):
                    xb = jax.device_put(x[d * per:(d + 1) * per], devs[d])
                    outs.append(f(xb, *weights[d]))
                res = np.concatenate([np.asarray(o) for o in outs], axis=0)
            else:
                res = np.asarray(f(*[jax.device_put(a, devs[0]) for a in args]))
        except Exception:
            res = np.asarray(f(*args))
        return res.astype(np.float32)
    except Exception:
        return _numpy_fallback(**inputs)
